# revision 56
# baseline (speedup 1.0000x reference)
"""Tensor-parallel multi-head attention (RoPE) kernel for 8 Trainium2 cores.

Shapes (hardcoded): x [2, 2048, 1024], 16 heads x head_dim 64.
Sharding: core c -> batch b = c//4, head-group hg = c%4 (4 heads = 256
projection columns). Each core computes q/k/v projections for its head
columns, RoPE, attention, and a partial out-projection over its 256 rows
of o_w; the host sums the 4 partials per batch and adds o_b (plus the
v_b @ o_w term, which passes through attention linearly).

Device-side layout choices:
  - qT/kT stored transposed [head_dim on partitions, tokens on free]
    so QK^T contracts over partitions directly.  Head dims are
    interleaved (0,32,1,33,...) so RoPE's rotate_half is an
    adjacent-partition swap = one DVE stream_shuffle (no DMAs); the
    host permutes q/k weight columns, biases and the cos/sin tables to
    match (scores are invariant to a shared head-dim permutation).
  - scores computed transposed S^T[k, q]; softmax max-subtraction is
    skipped (scores are O(+-6), fp32 exp is exact enough).
  - PV uses stationary [V | 1] so one accumulation produces both the
    unnormalized output and the softmax denominators (row 64).  The
    denominator row is copied to a partition-0 tile before
    reciprocal_approx_fast / partition_broadcast — both ucode ops
    silently misread inputs at a nonzero partition offset.
  - all matmuls run in bf16: on HW, fp32r streams moving columns at
    ~2 cycles/col while bf16 streams at 1 cycle/col, so bf16 halves PE
    busy time.  fp32 PSUM accumulation keeps the contraction exact.
  - DMA triggers cost ~650ns each on their issuing queue, so inputs are
    consolidated into one DMA per tensor (8 for x to pace the first
    projection) and spread across the sync/scalar/gpsimd/vector queues.
  - attention inner loop is software-pipelined: S(tk) and exp(tk) issue
    one k-tile ahead of PV(tk-1), so the PE never waits a full exp
    latency per k-tile.
  - scalar engine does only `exp` (+ half the output copies at the tail);
    everything else lands on the vector engine.
"""

import sys
import numpy as np

for p in ("/opt/trn_rl_repo", "/root/.axon_site/_ro/trn_rl_repo"):
    if p not in sys.path:
        sys.path.insert(0, p)

B, L, D = 2, 2048, 1024
H, HD = 16, 64
NCORES = 8
HG = 4                  # head-groups == cores per batch
EL = D // HG            # 256 projection columns per core
ET = EL // 128          # 2 e-tiles
DT = D // 128           # 8 d-tiles
TT = L // 128           # 16 token tiles
NH = H // HG            # 4 heads per core

_cache = {}

def _build():
    import concourse.mybir as mybir
    from concourse import bacc, tile

    F32 = mybir.dt.float32
    BF16 = mybir.dt.bfloat16
    AF = mybir.ActivationFunctionType

    nc = bacc.Bacc("TRN2", target_bir_lowering=False, debug=False,
                   num_devices=NCORES)

    xc = nc.dram_tensor("xc", [128, DT * L], BF16, kind="ExternalInput").ap()
    wq = nc.dram_tensor("wq", [128, DT * EL], BF16, kind="ExternalInput").ap()
    wk = nc.dram_tensor("wk", [128, DT * EL], BF16, kind="ExternalInput").ap()
    wv = nc.dram_tensor("wv", [128, DT * EL], BF16, kind="ExternalInput").ap()
    wo = nc.dram_tensor("wo", [128, ET * D], BF16, kind="ExternalInput").ap()
    bqk = nc.dram_tensor("bqk", [128, 2 * ET], F32, kind="ExternalInput").ap()
    cosb = nc.dram_tensor("cosb", [128, L], BF16, kind="ExternalInput").ap()
    sinb = nc.dram_tensor("sinb", [128, L], BF16, kind="ExternalInput").ap()
    onesc = nc.dram_tensor("onesc", [128, TT * NH], BF16,
                           kind="ExternalInput").ap()
    outT = nc.dram_tensor("outT", [D, L], BF16, kind="ExternalOutput").ap()

    with tile.TileContext(nc) as tc:
        with tc.tile_pool(name="persist", bufs=1) as P:
            qT = [P.tile([128, L], BF16, name=f"qT{e}") for e in range(ET)]
            kT = [P.tile([128, L], BF16, name=f"kT{e}") for e in range(ET)]
            Vsb = P.tile([128, TT * NH * 65], BF16, name="Vsb")
            ao = [P.tile([128, L], BF16, name=f"ao{e}") for e in range(ET)]

            xts = P.tile([128, DT * L], BF16, name="xts")
            wqs = P.tile([128, DT * EL], BF16, name="wqs")
            wks = P.tile([128, DT * EL], BF16, name="wks")
            wvs = P.tile([128, DT * EL], BF16, name="wvs")
            cosbt = P.tile([128, L], BF16, name="cosbt")
            sinbt = P.tile([128, L], BF16, name="sinbt")
            bqkt = P.tile([128, 2 * ET], F32, name="bqkt")
            wos = P.tile([128, ET * D], BF16, name="wos")

            # ---- input DMAs, spread across queues -----------------------
            # sync queue: x (per d-tile, paces the first projections)
            for d in range(DT):
                nc.sync.dma_start(xts[:, d * L:(d + 1) * L],
                                  xc[:, d * L:(d + 1) * L])
            # scalar queue: q/k weights are laid out e-major on the host so
            # each e-tile's weights are one contiguous DMA; the e0 halves
            # land first and gate only the e0 projection wave.
            HW_ = DT * EL // 2
            nc.scalar.dma_start(wqs[:, :HW_], wq[:, :HW_])
            nc.scalar.dma_start(wks[:, :HW_], wk[:, :HW_])
            nc.scalar.dma_start(wqs[:, HW_:], wq[:, HW_:])
            nc.scalar.dma_start(wks[:, HW_:], wk[:, HW_:])
            nc.scalar.dma_start(wvs[:], wv[:])
            # rope tables (needed ~33us) and out-proj weights (needed at the
            # tail) queue behind the critical weights so they don't steal
            # HBM bandwidth from the DMA-bound projection start.
            nc.scalar.dma_start(cosbt[:], cosb[:])
            nc.scalar.dma_start(sinbt[:], sinb[:])
            nc.scalar.dma_start(wos[:], wo[:])
            # gpsimd queue: biases + ones columns (tiny)
            nc.gpsimd.dma_start(bqkt[:], bqk[:])
            nc.gpsimd.dma_start(
                Vsb[:].rearrange("p (t c) -> p t c", c=65)[:, :, 64:65],
                onesc[:].rearrange("p (t o) -> p t o", o=1))

            def xck(d, c, n=512):
                return xts[:, d * L + c:d * L + c + n]

            def wslice(w, d, e):
                # e-major layout for q/k weights: [e-block][d-block]
                return w[:, (e * DT + d) * 128:(e * DT + d + 1) * 128]

            # adjacent-pair swap: rotate_half under the interleaved layout
            shuf_mask = [i ^ 1 for i in range(32)]

            # ---------------- Phase B: q/k/v projections + RoPE ----------
            # d-major wave over all four (q/k x column-half) groups of an
            # e-tile at once (4 psum tiles = all 8 banks): each arriving x
            # d-tile feeds 8 matmuls (~1.7us) >= its ~1.4us DMA time, so
            # the PE tracks the DMA with no per-half re-walk stalls.
            SS = tc.alloc_tile_pool(name="smallsb", bufs=2)
            with (
                tc.tile_pool(name="pb", bufs=1, space="PSUM") as PB,
                tc.tile_pool(name="ropet", bufs=2) as RT,
                tc.tile_pool(name="ebh", bufs=3) as EBH,
            ):
                HL = L // 2
                for e in range(ET):
                    pst = {}
                    for t, hf in ((0, 0), (1, 0), (0, 1), (1, 1)):
                        pst[(t, hf)] = PB.tile([128, HL], F32,
                                               name=f"ps{t}{hf}",
                                               tag=f"ps{t}{hf}")
                    for d in range(DT):
                        # q halves first: the first d-step needn't gate on
                        # the wk DMA landing
                        for t, hf in ((0, 0), (0, 1), (1, 0), (1, 1)):
                            w = wqs if t == 0 else wks
                            for c in range(0, HL, 512):
                                nc.tensor.matmul(
                                    pst[(t, hf)][:, c:c + 512],
                                    wslice(w, d, e), xck(d, hf * HL + c),
                                    start=(d == 0), stop=(d == DT - 1),
                                    skip_group_check=True)
                    # extraction: ACT does bias+cast, DVE the shuffle,
                    # cos/sin muls and add.
                    for t, hf in ((0, 0), (1, 0), (0, 1), (1, 1)):
                        q0 = hf * HL
                        dstf = qT[e] if t == 0 else kT[e]
                        bcol = e if t == 0 else ET + e
                        dst = dstf[:, q0:q0 + HL]
                        nc.scalar.activation(dst, pst[(t, hf)][:],
                                             AF.Identity,
                                             bias=bqkt[:, bcol:bcol + 1])
                        rs = RT.tile([128, HL], BF16, tag="rs")
                        tmp = RT.tile([128, HL], BF16, tag="tmp")
                        nc.vector.stream_shuffle(rs[:], dst, shuf_mask)
                        nc.vector.tensor_mul(tmp[:], dst,
                                             cosbt[:, q0:q0 + HL])
                        nc.vector.tensor_mul(rs[:], rs[:],
                                             sinbt[:, q0:q0 + HL])
                        nc.vector.tensor_add(dst, tmp[:], rs[:])

                # The V-wave carries head 0's entire first q-half with it:
                # V groups rotate through ONE tag (the ACT copy returns
                # well within a group's PE time, so one buffer suffices),
                # freeing ps10 as a persistent PV accumulator and ps01/ps11
                # as a 2-deep score runway.  The scalar engine — otherwise
                # ~70% idle during the V-wave — alternates V copies with
                # the first 16 exps, and the PE interleaves V, S and PV at
                # ~1.7us per k-tile instead of running the same work as two
                # serial phases.
                q00 = qT[0][0:64, :]
                k00 = kT[0][0:64, :]
                op0f = PB.tile([128, HL], F32, name="op0", tag="ps10")
                op0 = op0f[0:65, :]
                ebh = {}
                for t in range(TT + 1):
                    if t < TT:
                        ps = PB.tile([128, HL], F32, name=f"psv{t}",
                                     tag="ps00")
                        for d in range(DT):
                            nc.tensor.matmul(
                                ps[:, 0:EL], xck(d, t * 128, 128),
                                wvs[:, d * EL:(d + 1) * EL],
                                start=(d == 0), stop=(d == DT - 1),
                                skip_group_check=True)
                        dv = Vsb[:, t * NH * 65:(t + 1) * NH * 65].rearrange(
                            "p (h c) -> p h c", c=65)
                        nc.scalar.activation(
                            dv[:, :, 0:64],
                            ps[:, 0:EL].rearrange("p (h c) -> p h c", c=64),
                            AF.Identity)
                        sph = PB.tile([128, HL], F32, name=f"sph{t}",
                                      tag=("ps01" if t % 2 == 0 else "ps11"))
                        for c in range(0, HL, 512):
                            nc.tensor.matmul(
                                sph[:, c:c + 512],
                                k00[:, t * 128:(t + 1) * 128],
                                q00[:, c:c + 512],
                                start=True, stop=True,
                                skip_group_check=True)
                        eb = EBH.tile([128, HL], BF16, name=f"ebh{t}",
                                      tag="ebh")
                        nc.scalar.activation(eb[:], sph[:], AF.Exp,
                                             scale=0.125)
                        ebh[t] = eb
                    if t > 0:
                        tp = t - 1
                        eb = ebh.pop(tp)
                        for c in range(0, HL, 512):
                            nc.tensor.matmul(
                                op0[:, c:c + 512],
                                Vsb[:, tp * NH * 65:tp * NH * 65 + 65],
                                eb[:, c:c + 512],
                                start=(tp == 0), stop=(tp == TT - 1),
                                skip_group_check=True)
                # head 0 first-half normalize, emitted here so its op-PSUM
                # read is ordered before the attention pools alias the bank
                oraw = SS.tile([65, HL], F32, tag="oraw")
                nc.vector.tensor_copy(oraw[:], op0[:])
                dent = SS.tile([1, HL], F32, tag="dent")
                nc.vector.tensor_copy(dent[:], oraw[64:65, :])
                rb = SS.tile([1, HL], F32, tag="rb")
                nc.vector.reciprocal_approx_fast(rb[:], dent[0:1, :])
                rbB = SS.tile([64, HL], F32, tag="rbB")
                nc.gpsimd.partition_broadcast(rbB[:], rb[:], channels=64)
                nc.vector.tensor_mul(ao[0][0:64, 0:HL], oraw[0:64, :],
                                     rbB[:])

            # ---------------- Phase C: attention per head ----------------
            HL = L // 2
            with (
                tc.tile_pool(name="po", bufs=1, space="PSUM") as PO,
                tc.tile_pool(name="pscr", bufs=3, space="PSUM") as PS2,
                tc.tile_pool(name="esb", bufs=6) as EP,
            ):
                for h in range(NH):
                    e, off = divmod(h, 2)
                    off *= 64
                    qh = qT[e][off:off + 64, :]
                    kh = kT[e][off:off + 64, :]
                    for qf in range(2):
                        if h == 0 and qf == 0:
                            continue  # fully absorbed into the V-wave
                        q0 = qf * HL
                        op = PO.tile([65, HL], F32, tag="op")
                        ebs = {}
                        # software pipelined: S/exp for tk issue ahead of PV
                        # for tk-1, so the exp latency overlaps PV+S matmuls
                        # instead of sitting on the critical path.  3 S-psum
                        # buffers give the PE a deeper runway than the
                        # previous full-row layout allowed.
                        for tk in range(TT + 1):
                            if tk < TT:
                                sp = PS2.tile([128, HL], F32, tag="scr")
                                for c in range(0, HL, 512):
                                    nc.tensor.matmul(
                                        sp[:, c:c + 512],
                                        kh[:, tk * 128:(tk + 1) * 128],
                                        qh[:, q0 + c:q0 + c + 512],
                                        start=True, stop=True,
                                        skip_group_check=True)
                                eb = EP.tile([128, HL], BF16, tag="eb")
                                nc.scalar.activation(eb[:], sp[:], AF.Exp,
                                                     scale=0.125)
                                ebs[tk] = eb
                            if tk > 0:
                                tp = tk - 1
                                eb = ebs.pop(tp)
                                for c in range(0, HL, 512):
                                    nc.tensor.matmul(
                                        op[:, c:c + 512],
                                        Vsb[:, tp * NH * 65 + h * 65:
                                            tp * NH * 65 + h * 65 + 65],
                                        eb[:, c:c + 512],
                                        start=(tp == 0), stop=(tp == TT - 1),
                                        skip_group_check=True)
                        # Move raw output+denominators to SBUF immediately so
                        # the PSUM accumulator frees for the next half while
                        # the normalize runs on DVE fully overlapped with the
                        # next half's matmuls.
                        oraw = SS.tile([65, HL], F32, tag="oraw")
                        nc.vector.tensor_copy(oraw[:], op[:])
                        dent = SS.tile([1, HL], F32, tag="dent")
                        nc.vector.tensor_copy(dent[:], oraw[64:65, :])
                        rb = SS.tile([1, HL], F32, tag="rb")
                        nc.vector.reciprocal_approx_fast(rb[:], dent[0:1, :])
                        rbB = SS.tile([64, HL], F32, tag="rbB")
                        nc.gpsimd.partition_broadcast(rbB[:], rb[:],
                                                      channels=64)
                        nc.vector.tensor_mul(
                            ao[e][off:off + 64, q0:q0 + HL],
                            oraw[0:64, :], rbB[:])

                # ------------- Phase D: partial out-projection -----------
                # q-half-major so the first half starts as soon as the last
                # head's normalize covers columns 0:1024.  PSUM tiles come
                # from the score pool (same shape, tag-level WAR against
                # long-completed exps) instead of a fresh pool whose
                # allocation would barrier on the final oraw copy.
                with tc.tile_pool(name="od", bufs=4) as OD:
                    for qh in range(2):
                        q0 = qh * (L // 2)
                        for dc in range(DT):
                            pdt = PS2.tile([128, L // 2], F32, name="pdt",
                                           tag="scr")
                            for e in range(ET):
                                for c in range(0, L // 2, 512):
                                    nc.tensor.matmul(
                                        pdt[:, c:c + 512],
                                        wos[:, e * D + dc * 128:
                                            e * D + (dc + 1) * 128],
                                        ao[e][:, q0 + c:q0 + c + 512],
                                        start=(e == 0), stop=(e == ET - 1),
                                        skip_group_check=True)
                            osb = OD.tile([128, L // 2], BF16, tag="osb")
                            # copies alternate ACT/DVE so neither engine
                            # paces the tail; triggers go to the idle sync
                            # queue (issuing from scalar costs ACT time
                            # exactly where it is doing copies).
                            if dc % 2 == 0:
                                nc.scalar.activation(osb[:], pdt[:],
                                                     AF.Identity)
                            else:
                                nc.vector.tensor_copy(osb[:], pdt[:])
                            nc.sync.dma_start(
                                outT[dc * 128:(dc + 1) * 128,
                                     q0:q0 + L // 2],
                                osb[:])

            SS.release()

    nc.compile()
    return nc


# head-dim interleave: device partition 2j holds dim j, 2j+1 holds dim j+32
PERM64 = np.stack([np.arange(32), np.arange(32, 64)], 1).reshape(-1)


def _rope_tables():
    inv = 1.0 / (10000.0 ** (np.arange(0, HD, 2, dtype=np.float32) / HD))
    t = np.arange(L, dtype=np.float32)
    fr = t[:, None] * inv[None, :]                    # [L, 32]
    emb = np.concatenate([fr, fr], axis=1)            # [L, 64]
    cos, sin = np.cos(emb), np.sin(emb)               # [L, 64]
    # device layout [128, L]: row r covers head-dim PERM64[r % 64], two
    # heads stacked per 128-partition tile; sin carries the rotate_half
    # sign of the swapped-in source.
    i = PERM64[np.arange(128) % HD]
    cosb = cos.T[i, :]                                # [128, L]
    sg = np.where(i < HD // 2, -1.0, 1.0).astype(np.float32)
    sinb = sin.T[i, :] * sg[:, None]
    return np.ascontiguousarray(cosb, np.float32), \
        np.ascontiguousarray(sinb, np.float32)


def _perm_heads(a):
    # permute the last axis (multiple of 64, per-head blocks) by PERM64
    cols = a.shape[-1]
    v = a.reshape(*a.shape[:-1], cols // HD, HD)[..., PERM64]
    return v.reshape(*a.shape[:-1], cols)


def _dtile_layout(wT):
    # [D, cols] -> [128, DT*cols]: row p, block d holds wT[d*128+p, :]
    cols = wT.shape[1]
    return np.ascontiguousarray(
        wT.reshape(DT, 128, cols).transpose(1, 0, 2).reshape(128, DT * cols))


def _edtile_layout(wT):
    # [D, EL] -> [128, ET*DT*128], e-major: block (e, d) holds
    # wT[d*128+p, e*128:(e+1)*128] — each e-tile's weights contiguous so
    # one DMA covers exactly what one projection wave needs
    return np.ascontiguousarray(
        wT.reshape(DT, 128, ET, 128).transpose(1, 2, 0, 3)
        .reshape(128, ET * DT * 128))


def _in_maps(x, q_w, q_b, k_w, k_b, v_w, o_w):
    import ml_dtypes
    bf16 = ml_dtypes.bfloat16
    cosb, sinb = _rope_tables()
    cosb, sinb = cosb.astype(bf16), sinb.astype(bf16)
    qwT = _perm_heads(np.asarray(q_w, np.float32).T).astype(bf16)  # [D, D]
    kwT = _perm_heads(np.asarray(k_w, np.float32).T).astype(bf16)
    vwT = np.asarray(v_w, np.float32).T.astype(bf16)
    owT = np.asarray(o_w, np.float32).T.astype(bf16)   # [D(in rows), D]
    xTb = [np.asarray(x[b], np.float32).T.astype(bf16) for b in range(B)]
    xcb = [_dtile_layout(t) for t in xTb]
    maps = []
    for c in range(NCORES):
        b, hg = divmod(c, HG)
        er = slice(hg * EL, (hg + 1) * EL)
        woc = owT[er, :]                               # [EL, D]
        bq = _perm_heads(np.asarray(q_b, np.float32)[er]).reshape(ET, 128)
        bk = _perm_heads(np.asarray(k_b, np.float32)[er]).reshape(ET, 128)
        maps.append({
            "xc": xcb[b],
            "wq": _edtile_layout(qwT[:, er]),
            "wk": _edtile_layout(kwT[:, er]),
            "wv": _dtile_layout(vwT[:, er]),
            "wo": np.ascontiguousarray(
                woc.reshape(ET, 128, D).transpose(1, 0, 2).reshape(128, ET * D)),
            "bqk": np.ascontiguousarray(
                np.concatenate([bq, bk], 0).T),        # [128, 2*ET]
            "cosb": cosb,
            "sinb": sinb,
            "onesc": np.ones((128, TT * NH), bf16),
        })
    return maps


def kernel(x, q_w, q_b, k_w, k_b, v_w, v_b, o_w, o_b):
    from concourse.bass_utils import run_bass_kernel_spmd

    x = np.asarray(x, np.float32)
    assert x.shape == (B, L, D), x.shape

    if "nc" not in _cache:
        _cache["nc"] = _build()
    nc = _cache["nc"]

    in_maps = _in_maps(x, q_w, q_b, k_w, k_b, v_w, o_w)
    res = run_bass_kernel_spmd(nc, in_maps, list(range(NCORES)))

    out = np.zeros((B, L, D), np.float32)
    for c in range(NCORES):
        b = c // HG
        out[b] += res.results[c]["outT"].T.astype(np.float32)
    # o_b, plus v_b's contribution (v_b flows through softmax-weighted
    # averaging unchanged, then through the out-projection).
    extra = np.asarray(o_b, np.float32) + \
        np.asarray(v_b, np.float32) @ np.asarray(o_w, np.float32).T
    out += extra[None, None, :]
    return out


# revision 57
# speedup vs baseline: 1.1253x; 1.1253x over previous
"""Tensor-parallel multi-head attention (RoPE) kernel for 8 Trainium2 cores.

Shapes (hardcoded): x [2, 2048, 1024], 16 heads x head_dim 64.
Sharding: core c -> batch b = c//4, head-group hg = c%4 (4 heads = 256
projection columns). Each core computes q/k/v projections for its head
columns, RoPE, attention, and a partial out-projection over its 256 rows
of o_w; the host sums the 4 partials per batch and adds o_b (plus the
v_b @ o_w term, which passes through attention linearly).

Device-side layout choices:
  - qT/kT stored transposed [head_dim on partitions, tokens on free]
    so QK^T contracts over partitions directly.  Head dims are
    interleaved (0,32,1,33,...) so RoPE's rotate_half is an
    adjacent-partition swap = one DVE stream_shuffle (no DMAs); the
    host permutes q/k weight columns, biases and the cos/sin tables to
    match (scores are invariant to a shared head-dim permutation).
  - scores computed transposed S^T[k, q]; softmax max-subtraction is
    skipped (scores are O(+-6), fp32 exp is exact enough).
  - PV uses stationary [V | 1] so one accumulation produces both the
    unnormalized output and the softmax denominators (row 64).  The
    denominator row is copied to a partition-0 tile before
    reciprocal_approx_fast / partition_broadcast — both ucode ops
    silently misread inputs at a nonzero partition offset.
  - all matmuls run in bf16: on HW, fp32r streams moving columns at
    ~2 cycles/col while bf16 streams at 1 cycle/col, so bf16 halves PE
    busy time.  fp32 PSUM accumulation keeps the contraction exact.
  - DMA triggers cost ~650ns each on their issuing queue, so inputs are
    consolidated into one DMA per tensor (8 for x to pace the first
    projection) and spread across the sync/scalar/gpsimd/vector queues.
  - attention inner loop is software-pipelined: S(tk) and exp(tk) issue
    one k-tile ahead of PV(tk-1), so the PE never waits a full exp
    latency per k-tile.
  - scalar engine does only `exp` (+ half the output copies at the tail);
    everything else lands on the vector engine.
"""

import sys
import numpy as np

for p in ("/opt/trn_rl_repo", "/root/.axon_site/_ro/trn_rl_repo"):
    if p not in sys.path:
        sys.path.insert(0, p)

B, L, D = 2, 2048, 1024
H, HD = 16, 64
NCORES = 8
HG = 4                  # head-groups == cores per batch
EL = D // HG            # 256 projection columns per core
ET = EL // 128          # 2 e-tiles
DT = D // 128           # 8 d-tiles
TT = L // 128           # 16 token tiles
NH = H // HG            # 4 heads per core

_cache = {}

def _build():
    import concourse.mybir as mybir
    from concourse import bacc, tile

    F32 = mybir.dt.float32
    BF16 = mybir.dt.bfloat16
    AF = mybir.ActivationFunctionType

    nc = bacc.Bacc("TRN2", target_bir_lowering=False, debug=False,
                   num_devices=NCORES)

    xc = nc.dram_tensor("xc", [128, DT * L], BF16, kind="ExternalInput").ap()
    wq = nc.dram_tensor("wq", [128, DT * EL], BF16, kind="ExternalInput").ap()
    wk = nc.dram_tensor("wk", [128, DT * EL], BF16, kind="ExternalInput").ap()
    wv = nc.dram_tensor("wv", [128, DT * EL], BF16, kind="ExternalInput").ap()
    wo = nc.dram_tensor("wo", [128, ET * D], BF16, kind="ExternalInput").ap()
    bqk = nc.dram_tensor("bqk", [128, 2 * ET], F32, kind="ExternalInput").ap()
    cosb = nc.dram_tensor("cosb", [128, L], BF16, kind="ExternalInput").ap()
    sinb = nc.dram_tensor("sinb", [128, L], BF16, kind="ExternalInput").ap()
    onesc = nc.dram_tensor("onesc", [128, TT * NH], BF16,
                           kind="ExternalInput").ap()
    outT = nc.dram_tensor("outT", [D, L], BF16, kind="ExternalOutput").ap()

    with tile.TileContext(nc) as tc:
        with tc.tile_pool(name="persist", bufs=1) as P:
            qT = [P.tile([128, L], BF16, name=f"qT{e}") for e in range(ET)]
            kT = [P.tile([128, L], BF16, name=f"kT{e}") for e in range(ET)]
            Vsb = P.tile([128, TT * NH * 65], BF16, name="Vsb")
            ao = [P.tile([128, L], BF16, name=f"ao{e}") for e in range(ET)]

            xts = P.tile([128, DT * L], BF16, name="xts")
            wqs = P.tile([128, DT * EL], BF16, name="wqs")
            wks = P.tile([128, DT * EL], BF16, name="wks")
            wvs = P.tile([128, DT * EL], BF16, name="wvs")
            cosbt = P.tile([128, L], BF16, name="cosbt")
            sinbt = P.tile([128, L], BF16, name="sinbt")
            bqkt = P.tile([128, 2 * ET], F32, name="bqkt")
            wos = P.tile([128, ET * D], BF16, name="wos")

            # ---- input DMAs, spread across queues -----------------------
            # sync queue: x (per d-tile, paces the first projections)
            for d in range(DT):
                nc.sync.dma_start(xts[:, d * L:(d + 1) * L],
                                  xc[:, d * L:(d + 1) * L])
            # scalar queue: q/k weights are laid out e-major on the host so
            # each e-tile's weights are one contiguous DMA; the e0 halves
            # land first and gate only the e0 projection wave.
            HW_ = DT * EL // 2
            nc.scalar.dma_start(wqs[:, :HW_], wq[:, :HW_])
            nc.scalar.dma_start(wks[:, :HW_], wk[:, :HW_])
            nc.scalar.dma_start(wqs[:, HW_:], wq[:, HW_:])
            nc.scalar.dma_start(wks[:, HW_:], wk[:, HW_:])
            nc.scalar.dma_start(wvs[:], wv[:])
            # rope tables (needed ~33us) and out-proj weights (needed at the
            # tail) queue behind the critical weights so they don't steal
            # HBM bandwidth from the DMA-bound projection start.
            nc.scalar.dma_start(cosbt[:], cosb[:])
            nc.scalar.dma_start(sinbt[:], sinb[:])
            nc.scalar.dma_start(wos[:], wo[:])
            # gpsimd queue: biases + ones columns (tiny)
            nc.gpsimd.dma_start(bqkt[:], bqk[:])
            nc.gpsimd.dma_start(
                Vsb[:].rearrange("p (t c) -> p t c", c=65)[:, :, 64:65],
                onesc[:].rearrange("p (t o) -> p t o", o=1))

            def xck(d, c, n=512):
                return xts[:, d * L + c:d * L + c + n]

            def wslice(w, d, e):
                # e-major layout for q/k weights: [e-block][d-block]
                return w[:, (e * DT + d) * 128:(e * DT + d + 1) * 128]

            # adjacent-pair swap: rotate_half under the interleaved layout
            shuf_mask = [i ^ 1 for i in range(32)]

            # ---------------- Phase B: q/k/v projections + RoPE ----------
            # d-major wave over all four (q/k x column-half) groups of an
            # e-tile at once (4 psum tiles = all 8 banks): each arriving x
            # d-tile feeds 8 matmuls (~1.7us) >= its ~1.4us DMA time, so
            # the PE tracks the DMA with no per-half re-walk stalls.
            with (
                tc.tile_pool(name="pb", bufs=1, space="PSUM") as PB,
                tc.tile_pool(name="ropet", bufs=2) as RT,
            ):
                HL = L // 2
                for e in range(ET):
                    pst = {}
                    for t, hf in ((0, 0), (1, 0), (0, 1), (1, 1)):
                        pst[(t, hf)] = PB.tile([128, HL], F32,
                                               name=f"ps{t}{hf}",
                                               tag=f"ps{t}{hf}")
                    for d in range(DT):
                        # q halves first: the first d-step needn't gate on
                        # the wk DMA landing
                        for t, hf in ((0, 0), (0, 1), (1, 0), (1, 1)):
                            w = wqs if t == 0 else wks
                            for c in range(0, HL, 512):
                                nc.tensor.matmul(
                                    pst[(t, hf)][:, c:c + 512],
                                    wslice(w, d, e), xck(d, hf * HL + c),
                                    start=(d == 0), stop=(d == DT - 1),
                                    skip_group_check=True)
                    # extraction: ACT does bias+cast, DVE the shuffle,
                    # cos/sin muls and add.
                    for t, hf in ((0, 0), (1, 0), (0, 1), (1, 1)):
                        q0 = hf * HL
                        dstf = qT[e] if t == 0 else kT[e]
                        bcol = e if t == 0 else ET + e
                        dst = dstf[:, q0:q0 + HL]
                        nc.scalar.activation(dst, pst[(t, hf)][:],
                                             AF.Identity,
                                             bias=bqkt[:, bcol:bcol + 1])
                        rs = RT.tile([128, HL], BF16, tag="rs")
                        tmp = RT.tile([128, HL], BF16, tag="tmp")
                        nc.vector.stream_shuffle(rs[:], dst, shuf_mask)
                        nc.vector.tensor_mul(tmp[:], dst,
                                             cosbt[:, q0:q0 + HL])
                        nc.vector.tensor_mul(rs[:], rs[:],
                                             sinbt[:, q0:q0 + HL])
                        nc.vector.tensor_add(dst, tmp[:], rs[:])

                # V-projection reuses two of the wave tags (alternating) so
                # each V group's WAR waits only on that one tile's e1
                # extraction read, staggered — not a pool-close barrier
                # across all four extractions.
                for t in range(TT):
                    ps = PB.tile([128, HL], F32, name=f"psv{t}",
                                 tag=("ps00" if t % 2 == 0 else "ps10"))
                    for d in range(DT):
                        nc.tensor.matmul(
                            ps[:, 0:EL], xck(d, t * 128, 128),
                            wvs[:, d * EL:(d + 1) * EL],
                            start=(d == 0), stop=(d == DT - 1),
                            skip_group_check=True)
                    dv = Vsb[:, t * NH * 65:(t + 1) * NH * 65].rearrange(
                        "p (h c) -> p h c", c=65)
                    # ACT is idle until the first attention exp
                    nc.scalar.activation(
                        dv[:, :, 0:64],
                        ps[:, 0:EL].rearrange("p (h c) -> p h c", c=64),
                        AF.Identity)

            # ---------------- Phase C: attention per head ----------------
            HL = L // 2
            with (
                tc.tile_pool(name="po", bufs=1, space="PSUM") as PO,
                tc.tile_pool(name="pscr", bufs=3, space="PSUM") as PS2,
                tc.tile_pool(name="esb", bufs=6) as EP,
                tc.tile_pool(name="smallsb", bufs=2) as SS,
            ):
                for h in range(NH):
                    e, off = divmod(h, 2)
                    off *= 64
                    qh = qT[e][off:off + 64, :]
                    kh = kT[e][off:off + 64, :]
                    for qf in range(2):
                        q0 = qf * HL
                        op = PO.tile([65, HL], F32, tag="op")
                        ebs = {}
                        # software pipelined: S/exp for tk issue ahead of PV
                        # for tk-1, so the exp latency overlaps PV+S matmuls
                        # instead of sitting on the critical path.  3 S-psum
                        # buffers give the PE a deeper runway than the
                        # previous full-row layout allowed.
                        for tk in range(TT + 1):
                            if tk < TT:
                                sp = PS2.tile([128, HL], F32, tag="scr")
                                for c in range(0, HL, 512):
                                    nc.tensor.matmul(
                                        sp[:, c:c + 512],
                                        kh[:, tk * 128:(tk + 1) * 128],
                                        qh[:, q0 + c:q0 + c + 512],
                                        start=True, stop=True,
                                        skip_group_check=True)
                                eb = EP.tile([128, HL], BF16, tag="eb")
                                nc.scalar.activation(eb[:], sp[:], AF.Exp,
                                                     scale=0.125)
                                ebs[tk] = eb
                            if tk > 0:
                                tp = tk - 1
                                eb = ebs.pop(tp)
                                for c in range(0, HL, 512):
                                    nc.tensor.matmul(
                                        op[:, c:c + 512],
                                        Vsb[:, tp * NH * 65 + h * 65:
                                            tp * NH * 65 + h * 65 + 65],
                                        eb[:, c:c + 512],
                                        start=(tp == 0), stop=(tp == TT - 1),
                                        skip_group_check=True)
                        # Move raw output+denominators to SBUF immediately so
                        # the PSUM accumulator frees for the next half while
                        # the normalize runs on DVE fully overlapped with the
                        # next half's matmuls.
                        oraw = SS.tile([65, HL], F32, tag="oraw")
                        nc.vector.tensor_copy(oraw[:], op[:])
                        dent = SS.tile([1, HL], F32, tag="dent")
                        nc.vector.tensor_copy(dent[:], oraw[64:65, :])
                        rb = SS.tile([1, HL], F32, tag="rb")
                        nc.vector.reciprocal_approx_fast(rb[:], dent[0:1, :])
                        rbB = SS.tile([64, HL], F32, tag="rbB")
                        nc.gpsimd.partition_broadcast(rbB[:], rb[:],
                                                      channels=64)
                        nc.vector.tensor_mul(
                            ao[e][off:off + 64, q0:q0 + HL],
                            oraw[0:64, :], rbB[:])

                # ------------- Phase D: partial out-projection -----------
                # q-half-major so the first half starts as soon as the last
                # head's normalize covers columns 0:1024.  PSUM tiles come
                # from the score pool (same shape, tag-level WAR against
                # long-completed exps) instead of a fresh pool whose
                # allocation would barrier on the final oraw copy.
                with tc.tile_pool(name="od", bufs=4) as OD:
                    for qh in range(2):
                        q0 = qh * (L // 2)
                        for dc in range(DT):
                            pdt = PS2.tile([128, L // 2], F32, name="pdt",
                                           tag="scr")
                            for e in range(ET):
                                for c in range(0, L // 2, 512):
                                    nc.tensor.matmul(
                                        pdt[:, c:c + 512],
                                        wos[:, e * D + dc * 128:
                                            e * D + (dc + 1) * 128],
                                        ao[e][:, q0 + c:q0 + c + 512],
                                        start=(e == 0), stop=(e == ET - 1),
                                        skip_group_check=True)
                            osb = OD.tile([128, L // 2], BF16, tag="osb")
                            # copies alternate ACT/DVE so neither engine
                            # paces the tail; triggers go to the idle sync
                            # queue (issuing from scalar costs ACT time
                            # exactly where it is doing copies).
                            if dc % 2 == 0:
                                nc.scalar.activation(osb[:], pdt[:],
                                                     AF.Identity)
                            else:
                                nc.vector.tensor_copy(osb[:], pdt[:])
                            nc.sync.dma_start(
                                outT[dc * 128:(dc + 1) * 128,
                                     q0:q0 + L // 2],
                                osb[:])

    nc.compile()
    return nc


# head-dim interleave: device partition 2j holds dim j, 2j+1 holds dim j+32
PERM64 = np.stack([np.arange(32), np.arange(32, 64)], 1).reshape(-1)


def _rope_tables():
    inv = 1.0 / (10000.0 ** (np.arange(0, HD, 2, dtype=np.float32) / HD))
    t = np.arange(L, dtype=np.float32)
    fr = t[:, None] * inv[None, :]                    # [L, 32]
    emb = np.concatenate([fr, fr], axis=1)            # [L, 64]
    cos, sin = np.cos(emb), np.sin(emb)               # [L, 64]
    # device layout [128, L]: row r covers head-dim PERM64[r % 64], two
    # heads stacked per 128-partition tile; sin carries the rotate_half
    # sign of the swapped-in source.
    i = PERM64[np.arange(128) % HD]
    cosb = cos.T[i, :]                                # [128, L]
    sg = np.where(i < HD // 2, -1.0, 1.0).astype(np.float32)
    sinb = sin.T[i, :] * sg[:, None]
    return np.ascontiguousarray(cosb, np.float32), \
        np.ascontiguousarray(sinb, np.float32)


def _perm_heads(a):
    # permute the last axis (multiple of 64, per-head blocks) by PERM64
    cols = a.shape[-1]
    v = a.reshape(*a.shape[:-1], cols // HD, HD)[..., PERM64]
    return v.reshape(*a.shape[:-1], cols)


def _dtile_layout(wT):
    # [D, cols] -> [128, DT*cols]: row p, block d holds wT[d*128+p, :]
    cols = wT.shape[1]
    return np.ascontiguousarray(
        wT.reshape(DT, 128, cols).transpose(1, 0, 2).reshape(128, DT * cols))


def _edtile_layout(wT):
    # [D, EL] -> [128, ET*DT*128], e-major: block (e, d) holds
    # wT[d*128+p, e*128:(e+1)*128] — each e-tile's weights contiguous so
    # one DMA covers exactly what one projection wave needs
    return np.ascontiguousarray(
        wT.reshape(DT, 128, ET, 128).transpose(1, 2, 0, 3)
        .reshape(128, ET * DT * 128))


def _in_maps(x, q_w, q_b, k_w, k_b, v_w, o_w):
    import ml_dtypes
    bf16 = ml_dtypes.bfloat16
    cosb, sinb = _rope_tables()
    cosb, sinb = cosb.astype(bf16), sinb.astype(bf16)
    qwT = _perm_heads(np.asarray(q_w, np.float32).T).astype(bf16)  # [D, D]
    kwT = _perm_heads(np.asarray(k_w, np.float32).T).astype(bf16)
    vwT = np.asarray(v_w, np.float32).T.astype(bf16)
    owT = np.asarray(o_w, np.float32).T.astype(bf16)   # [D(in rows), D]
    xTb = [np.asarray(x[b], np.float32).T.astype(bf16) for b in range(B)]
    xcb = [_dtile_layout(t) for t in xTb]
    maps = []
    for c in range(NCORES):
        b, hg = divmod(c, HG)
        er = slice(hg * EL, (hg + 1) * EL)
        woc = owT[er, :]                               # [EL, D]
        bq = _perm_heads(np.asarray(q_b, np.float32)[er]).reshape(ET, 128)
        bk = _perm_heads(np.asarray(k_b, np.float32)[er]).reshape(ET, 128)
        maps.append({
            "xc": xcb[b],
            "wq": _edtile_layout(qwT[:, er]),
            "wk": _edtile_layout(kwT[:, er]),
            "wv": _dtile_layout(vwT[:, er]),
            "wo": np.ascontiguousarray(
                woc.reshape(ET, 128, D).transpose(1, 0, 2).reshape(128, ET * D)),
            "bqk": np.ascontiguousarray(
                np.concatenate([bq, bk], 0).T),        # [128, 2*ET]
            "cosb": cosb,
            "sinb": sinb,
            "onesc": np.ones((128, TT * NH), bf16),
        })
    return maps


def kernel(x, q_w, q_b, k_w, k_b, v_w, v_b, o_w, o_b):
    from concourse.bass_utils import run_bass_kernel_spmd

    x = np.asarray(x, np.float32)
    assert x.shape == (B, L, D), x.shape

    if "nc" not in _cache:
        _cache["nc"] = _build()
    nc = _cache["nc"]

    in_maps = _in_maps(x, q_w, q_b, k_w, k_b, v_w, o_w)
    res = run_bass_kernel_spmd(nc, in_maps, list(range(NCORES)))

    out = np.zeros((B, L, D), np.float32)
    for c in range(NCORES):
        b = c // HG
        out[b] += res.results[c]["outT"].T.astype(np.float32)
    # o_b, plus v_b's contribution (v_b flows through softmax-weighted
    # averaging unchanged, then through the out-projection).
    extra = np.asarray(o_b, np.float32) + \
        np.asarray(v_b, np.float32) @ np.asarray(o_w, np.float32).T
    out += extra[None, None, :]
    return out
